# revision 1
# baseline (speedup 1.0000x reference)
"""Trainium2 Bass kernel for a 3-layer cross-attention decoder + final
single-head attention-score output.

Sharding: 8 cores = (4 batches) x (2 sequence halves). The reference's
head split is a plain row-major reshape, so each attention head only
touches a contiguous 128-row block of queries / 256-row block of keys.
The entire 3-layer stack is therefore embarrassingly parallel across
(batch, half) shards, and the final q@k^T output splits by query rows.
No collectives.

Per-core program (shard-local sizes): h rows 512, x rows 1024 (full x
2048 for the final stage), 4 local heads with per-head Q_=1024, K_=2048,
dh=64. Matmuls run in bf16 with fp32 PSUM accumulation; softmax
(exp/mask/denominator) and layernorms in fp32.
"""
import sys
import math

if '/opt/trn_rl_repo' not in sys.path:
    sys.path.insert(0, '/opt/trn_rl_repo')

import numpy as np
import ml_dtypes

BF16 = ml_dtypes.bfloat16

B, LQ, LK, D = 4, 1024, 2048, 512
NL, NHEAD, DH, FF = 3, 8, 64, 512
NCORES = 8
LQL, LKL = 512, 1024          # per-core local rows (h side, x side)
NHL = 4                       # local heads per core
QH, KH = 1024, 2048           # per-head q / k index ranges
NKT = KH // 128               # 16 k-tiles per head
LN_EPS = 1e-5

TRACE = False                 # test.py flips this for profiled runs
_PROG = None                  # cached (nc, input names)


def _build_program():
    import concourse.bass as bass
    import concourse.tile as tile
    from concourse import bacc, mybir
    from concourse.masks import make_identity

    F32 = mybir.dt.float32
    BF = mybir.dt.bfloat16
    AF = mybir.ActivationFunctionType

    nc = bacc.Bacc(None, target_bir_lowering=False, debug=False)

    # ---- per-core external inputs ----
    hT0 = nc.dram_tensor("hT0", [D, LQL], BF, kind="ExternalInput").ap()
    h0 = nc.dram_tensor("h0", [LQL, D], F32, kind="ExternalInput").ap()
    xTl = nc.dram_tensor("xTl", [D, LKL], BF, kind="ExternalInput").ap()
    xTf = nc.dram_tensor("xTf", [D, LK], BF, kind="ExternalInput").ap()
    nmT = nc.dram_tensor("nmT", [KH, QH], BF, kind="ExternalInput").ap()
    wq = nc.dram_tensor("wq", [NL, D, D], BF, kind="ExternalInput").ap()
    wk = nc.dram_tensor("wk", [NL, D, D], BF, kind="ExternalInput").ap()
    wv = nc.dram_tensor("wv", [NL, D, D], BF, kind="ExternalInput").ap()
    wo8 = nc.dram_tensor("wo8", [NL, 8, DH, D], BF, kind="ExternalInput").ap()
    fw1 = nc.dram_tensor("fw1", [NL, D, FF], BF, kind="ExternalInput").ap()
    fw2 = nc.dram_tensor("fw2", [NL, FF, D], BF, kind="ExternalInput").ap()
    vecs = nc.dram_tensor("vecs", [NL, 6, D], F32, kind="ExternalInput").ap()
    owq = nc.dram_tensor("owq", [D, D], BF, kind="ExternalInput").ap()
    owk = nc.dram_tensor("owk", [D, D], BF, kind="ExternalInput").ap()
    uout = nc.dram_tensor("u", [LQL, LK], F32, kind="ExternalOutput").ap()

    # ---- DRAM scratch ----
    qscr = nc.dram_tensor("qscr", [2, LQL, D], BF).ap()
    kscr = nc.dram_tensor("kscr", [2, LKL, D], BF).ap()
    vscr = nc.dram_tensor("vscr", [2, LKL, D], BF).ap()
    h1scr = nc.dram_tensor("h1scr", [2, LQL, D], BF).ap()
    midscr = nc.dram_tensor("midscr", [2, LQL, FF], BF).ap()
    hscr = nc.dram_tensor("hscr", [NL, LQL, D], BF).ap()

    # flat per-head views of q/k/v scratch: [rows, 512] -> [(rows*8), 64]
    qfl = [qscr[s].rearrange("r (j d) -> (r j) d", d=DH) for s in range(2)]
    kfl = [kscr[s].rearrange("r (j d) -> (r j) d", d=DH) for s in range(2)]
    vfl = [vscr[s].rearrange("r (j d) -> (r j) d", d=DH) for s in range(2)]

    IQ, IK, IV, IF1, IF2 = 0, 1, 2, 3, 4

    with tile.TileContext(nc) as tc:
        with tc.tile_pool(name="const", bufs=1) as cp:
            ident = cp.tile([128, 128], BF, tag="ident")
            make_identity(nc, ident)
            eps_t = cp.tile([128, 1], F32, tag="eps")
            nc.vector.memset(eps_t, LN_EPS)
            nm_sb = cp.tile([128, NKT, QH], BF, tag="nmT")
            nc.gpsimd.dma_start(out=nm_sb, in_=nmT.rearrange("(t p) q -> p t q", p=128))
            xTl_sb = cp.tile([128, 4, LKL], BF, tag="xTl")
            nc.gpsimd.dma_start(out=xTl_sb, in_=xTl.rearrange("(t p) r -> p t r", p=128))
            h_first = cp.tile([128, 4, D], F32, tag="h0")
            nc.gpsimd.dma_start(out=h_first, in_=h0.rearrange("(t p) d -> p t d", p=128))

            with tc.tile_pool(name="wts", bufs=1) as wp, \
                 tc.tile_pool(name="work", bufs=1) as w1, \
                 tc.tile_pool(name="work2", bufs=2) as w2, \
                 tc.tile_pool(name="work3", bufs=3) as w3, \
                 tc.tile_pool(name="ps_st", bufs=2, space="PSUM") as ps_st, \
                 tc.tile_pool(name="ps_oT", bufs=1, space="PSUM") as ps_oT, \
                 tc.tile_pool(name="ps_sm", bufs=2, space="PSUM") as ps_sm:

                h_sb = h_first
                for li in range(NL):
                    s = li % 2
                    # ---- layer weights ----
                    w_sb = wp.tile([128, 4, 5, D], BF, tag="w5")
                    for mi, wsrc in ((IQ, wq[li]), (IK, wk[li]), (IV, wv[li]),
                                     (IF1, fw1[li]), (IF2, fw2[li])):
                        nc.gpsimd.dma_start(
                            out=w_sb[:, :, mi, :],
                            in_=wsrc.rearrange("(t p) n -> p t n", p=128))
                    wo_sb = wp.tile([DH, 8, D], BF, tag="wo8")
                    nc.gpsimd.dma_start(out=wo_sb, in_=wo8[li].rearrange("j p n -> p j n"))
                    vrep = wp.tile([128, 6, D], F32, tag="vrep")
                    vsrc = vecs[li]
                    nc.gpsimd.dma_start(
                        out=vrep,
                        in_=bass.AP(tensor=vsrc.tensor, offset=vsrc.offset,
                                    ap=[[0, 128]] + list(vsrc.ap)))

                    # ---- transposed h for projections ----
                    hT = w1.tile([128, 4, LQL], BF, tag="hT")
                    if li == 0:
                        for t in range(4):
                            nc.sync.dma_start(out=hT[:, t, :], in_=hT0[t*128:(t+1)*128, :])
                    else:
                        for t in range(4):
                            nc.sync.dma_start_transpose(
                                out=hT[:, t, :], in_=hscr[li-1][:, t*128:(t+1)*128])

                    # ---- q/k/v projections -> row-layout scratch ----
                    for rt in range(4):
                        pp = ps_sm.tile([128, D], F32, tag="mm512")
                        for kt in range(4):
                            nc.tensor.matmul(pp, hT[:, kt, rt*128:(rt+1)*128],
                                             w_sb[:, kt, IQ, :],
                                             start=(kt == 0), stop=(kt == 3))
                        row = w3.tile([128, D], BF, tag="prow")
                        nc.scalar.activation(row, pp, AF.Copy)
                        nc.sync.dma_start(out=qscr[s][rt*128:(rt+1)*128, :], in_=row)
                    for mi, scr in ((IK, kscr), (IV, vscr)):
                        for rt in range(8):
                            pp = ps_sm.tile([128, D], F32, tag="mm512")
                            for kt in range(4):
                                nc.tensor.matmul(pp, xTl_sb[:, kt, rt*128:(rt+1)*128],
                                                 w_sb[:, kt, mi, :],
                                                 start=(kt == 0), stop=(kt == 3))
                            row = w3.tile([128, D], BF, tag="prow")
                            nc.scalar.activation(row, pp, AF.Copy)
                            nc.sync.dma_start(out=scr[s][rt*128:(rt+1)*128, :], in_=row)

                    ln1_sb = w1.tile([128, 4, D], F32, tag="ln1")

                    # ---- attention heads ----
                    for hh in range(NHL):
                        # per-head tiles in flat (k_, d) layout
                        qhf = w2.tile([128, 8, DH], BF, tag="qhf")
                        nc.gpsimd.dma_start(
                            out=qhf,
                            in_=qfl[s][hh*QH:(hh+1)*QH, :].rearrange("(t p) d -> p t d", p=128))
                        khf = w2.tile([128, NKT, DH], BF, tag="khf")
                        nc.gpsimd.dma_start(
                            out=khf,
                            in_=kfl[s][hh*KH:(hh+1)*KH, :].rearrange("(t p) d -> p t d", p=128))
                        vt = w2.tile([128, NKT, DH + 1], BF, tag="vt")
                        nc.gpsimd.dma_start(
                            out=vt[:, :, 0:DH],
                            in_=vfl[s][hh*KH:(hh+1)*KH, :].rearrange("(t p) d -> p t d", p=128))
                        nc.vector.memset(vt[:, :, DH:DH+1], 1.0)

                        # PE-transpose per-head Q,K into [dh, pos] layout
                        qhT = w2.tile([DH, QH], BF, tag="qhT")
                        for grp in range(2):
                            tp = ps_sm.tile([DH, 512], BF, tag="mm512")
                            for c in range(4):
                                nc.tensor.transpose(tp[:, c*128:(c+1)*128],
                                                    qhf[:, grp*4+c, :], ident)
                            nc.scalar.activation(qhT[:, grp*512:(grp+1)*512], tp, AF.Copy)
                        khT = w2.tile([DH, KH], BF, tag="khT")
                        for grp in range(4):
                            tp = ps_sm.tile([DH, 512], BF, tag="mm512")
                            for c in range(4):
                                nc.tensor.transpose(tp[:, c*128:(c+1)*128],
                                                    khf[:, grp*4+c, :], ident)
                            nc.scalar.activation(khT[:, grp*512:(grp+1)*512], tp, AF.Copy)

                        # scores^T -> exp -> mask -> @ [V|1]
                        oT = ps_oT.tile([DH + 1, QH], F32, tag="oT")
                        for kti in range(NKT):
                            st = ps_st.tile([128, QH], F32, tag="st")
                            for qn in range(2):
                                nc.tensor.matmul(st[:, qn*512:(qn+1)*512],
                                                 khT[:, kti*128:(kti+1)*128],
                                                 qhT[:, qn*512:(qn+1)*512],
                                                 start=True, stop=True)
                            pT = w3.tile([128, QH], BF, tag="pT")
                            nc.scalar.activation(pT, st, AF.Exp)
                            nc.vector.tensor_mul(pT, pT, nm_sb[:, kti, :])
                            for qn in range(2):
                                nc.tensor.matmul(oT[:, qn*512:(qn+1)*512],
                                                 vt[:, kti, :],
                                                 pT[:, qn*512:(qn+1)*512],
                                                 start=(kti == 0), stop=(kti == NKT - 1))

                        # normalize: oTn = oT[0:64] / denom (denom = row 64)
                        dn = w2.tile([1, QH], F32, tag="dn")
                        nc.scalar.activation(dn, oT[DH:DH+1, :], AF.Copy)
                        rd = w2.tile([1, QH], F32, tag="rd")
                        nc.vector.reciprocal(rd, dn)
                        rep = w1.tile([DH, QH], F32, tag="rep")
                        nc.gpsimd.partition_broadcast(rep, rd, channels=DH)
                        oTn = w2.tile([DH, QH], BF, tag="oTn")
                        nc.vector.tensor_mul(oTn, oT[0:DH, :], rep)

                        # w_o projection straight from oTn (j-strided lhsT)
                        aout = ps_sm.tile([128, D], F32, tag="mm512")
                        for j in range(8):
                            nc.tensor.matmul(aout, oTn[:, j::8], wo_sb[:, j, :],
                                             start=(j == 0), stop=(j == 7))

                        # residual + LN1 for this head's row block
                        t1 = w2.tile([128, D], F32, tag="t1")
                        nc.vector.tensor_add(t1, h_sb[:, hh, :], aout)
                        stats = w2.tile([128, 6], F32, tag="stats")
                        nc.vector.bn_stats(stats, t1)
                        mv = w2.tile([128, 2], F32, tag="mv")
                        nc.vector.bn_aggr(mv, stats)
                        sd = w2.tile([128, 1], F32, tag="sd")
                        nc.scalar.activation(sd, mv[:, 1:2], AF.Sqrt, bias=eps_t)
                        rs = w2.tile([128, 1], F32, tag="rs")
                        nc.vector.reciprocal(rs, sd)
                        t2 = w2.tile([128, D], F32, tag="t2")
                        nc.vector.tensor_scalar(t2, t1, scalar1=mv[:, 0:1], scalar2=rs,
                                                op0=mybir.AluOpType.subtract,
                                                op1=mybir.AluOpType.mult)
                        nc.vector.tensor_mul(t2, t2, vrep[:, 0, :])
                        nc.vector.tensor_add(ln1_sb[:, hh, :], t2, vrep[:, 1, :])
                        lb = w3.tile([128, D], BF, tag="prow")
                        nc.scalar.activation(lb, ln1_sb[:, hh, :], AF.Copy)
                        nc.sync.dma_start(out=h1scr[s][hh*128:(hh+1)*128, :], in_=lb)

                    # ---- FFN ----
                    h1T = w1.tile([128, 4, LQL], BF, tag="h1T")
                    for t in range(4):
                        nc.sync.dma_start_transpose(
                            out=h1T[:, t, :], in_=h1scr[s][:, t*128:(t+1)*128])
                    for rt in range(4):
                        p1 = ps_sm.tile([128, FF], F32, tag="mm512")
                        for kt in range(4):
                            nc.tensor.matmul(p1, h1T[:, kt, rt*128:(rt+1)*128],
                                             w_sb[:, kt, IF1, :],
                                             start=(kt == 0), stop=(kt == 3))
                        nc.vector.tensor_add(p1, p1, vrep[:, 4, :])
                        mid = w3.tile([128, FF], BF, tag="prow")
                        nc.scalar.activation(mid, p1, AF.Relu)
                        nc.sync.dma_start(out=midscr[s][rt*128:(rt+1)*128, :], in_=mid)
                    midT = w1.tile([128, 4, LQL], BF, tag="midT")
                    for t in range(4):
                        nc.sync.dma_start_transpose(
                            out=midT[:, t, :], in_=midscr[s][:, t*128:(t+1)*128])

                    h_new = w2.tile([128, 4, D], F32, tag="h")
                    for rt in range(4):
                        p2 = ps_sm.tile([128, D], F32, tag="mm512")
                        for kt in range(4):
                            nc.tensor.matmul(p2, midT[:, kt, rt*128:(rt+1)*128],
                                             w_sb[:, kt, IF2, :],
                                             start=(kt == 0), stop=(kt == 3))
                        nc.vector.tensor_add(p2, p2, vrep[:, 5, :])
                        t1 = w2.tile([128, D], F32, tag="t1")
                        nc.vector.tensor_add(t1, ln1_sb[:, rt, :], p2)
                        stats = w2.tile([128, 6], F32, tag="stats")
                        nc.vector.bn_stats(stats, t1)
                        mv = w2.tile([128, 2], F32, tag="mv")
                        nc.vector.bn_aggr(mv, stats)
                        sd = w2.tile([128, 1], F32, tag="sd")
                        nc.scalar.activation(sd, mv[:, 1:2], AF.Sqrt, bias=eps_t)
                        rs = w2.tile([128, 1], F32, tag="rs")
                        nc.vector.reciprocal(rs, sd)
                        t2 = w2.tile([128, D], F32, tag="t2")
                        nc.vector.tensor_scalar(t2, t1, scalar1=mv[:, 0:1], scalar2=rs,
                                                op0=mybir.AluOpType.subtract,
                                                op1=mybir.AluOpType.mult)
                        nc.vector.tensor_mul(t2, t2, vrep[:, 2, :])
                        nc.vector.tensor_add(h_new[:, rt, :], t2, vrep[:, 3, :])
                        hb = w3.tile([128, D], BF, tag="prow")
                        nc.scalar.activation(hb, h_new[:, rt, :], AF.Copy)
                        nc.sync.dma_start(out=hscr[li][rt*128:(rt+1)*128, :], in_=hb)
                    h_sb = h_new

            # ---- final stage: u = (h @ owq) @ (x @ owk)^T ----
            with tc.tile_pool(name="fin", bufs=1) as fp, \
                 tc.tile_pool(name="fin2", bufs=2) as fp2, \
                 tc.tile_pool(name="ps_fin", bufs=3, space="PSUM") as ps_f:
                xTf_sb = fp.tile([128, 4, LK], BF, tag="xTf")
                nc.gpsimd.dma_start(out=xTf_sb, in_=xTf.rearrange("(t p) r -> p t r", p=128))
                hTf = fp.tile([128, 4, LQL], BF, tag="hTf")
                for t in range(4):
                    nc.sync.dma_start_transpose(
                        out=hTf[:, t, :], in_=hscr[NL-1][:, t*128:(t+1)*128])
                owq_sb = fp.tile([128, 4, D], BF, tag="owq")
                nc.gpsimd.dma_start(out=owq_sb, in_=owq.rearrange("(t p) n -> p t n", p=128))
                owk_sb = fp.tile([128, 4, D], BF, tag="owk")
                nc.gpsimd.dma_start(out=owk_sb, in_=owk.rearrange("(t p) n -> p t n", p=128))

                qfT = fp.tile([128, 4, LQL], BF, tag="qfT")
                for dt in range(4):
                    pq = ps_f.tile([128, LQL], F32, tag="fmm")
                    for kt in range(4):
                        nc.tensor.matmul(pq, owq_sb[:, kt, dt*128:(dt+1)*128],
                                         hTf[:, kt, :], start=(kt == 0), stop=(kt == 3))
                    nc.scalar.activation(qfT[:, dt, :], pq, AF.Copy)
                kfT = fp.tile([128, 4, LK], BF, tag="kfT")
                for dt in range(4):
                    for ch in range(4):
                        pk = ps_f.tile([128, 512], F32, tag="fmm")
                        for kt in range(4):
                            nc.tensor.matmul(pk, owk_sb[:, kt, dt*128:(dt+1)*128],
                                             xTf_sb[:, kt, ch*512:(ch+1)*512],
                                             start=(kt == 0), stop=(kt == 3))
                        nc.scalar.activation(kfT[:, dt, ch*512:(ch+1)*512], pk, AF.Copy)
                for rt in range(4):
                    for ch in range(4):
                        pu = ps_f.tile([128, 512], F32, tag="fmm")
                        for kt in range(4):
                            nc.tensor.matmul(pu, qfT[:, kt, rt*128:(rt+1)*128],
                                             kfT[:, kt, ch*512:(ch+1)*512],
                                             start=(kt == 0), stop=(kt == 3))
                        us = fp2.tile([128, 512], F32, tag="us")
                        nc.vector.tensor_copy(us, pu)
                        nc.sync.dma_start(out=uout[rt*128:(rt+1)*128, ch*512:(ch+1)*512],
                                          in_=us)
    nc.compile()
    return nc


def _get_program():
    global _PROG
    if _PROG is None:
        _PROG = _build_program()
    return _PROG


def shard_inputs(x, h, mask, wq, wk, wv, wo, ffw1, ffb1, ffw2, ffb2,
                 g1, be1, g2, be2, out_wq, out_wk):
    """Host-side prep: returns list of 8 per-core input dicts."""
    f32 = np.float32
    x = np.asarray(x, f32)
    h = np.asarray(h, f32)
    mask = np.asarray(mask)
    bf = lambda a: np.ascontiguousarray(np.asarray(a, f32), dtype=f32).astype(BF16)

    wq_s = np.asarray(wq, f32) / math.sqrt(DH)      # fold 1/sqrt(dh) into Wq
    wo8 = np.asarray(wo, f32).reshape(NL, 8, DH, D)
    owq_s = np.asarray(out_wq, f32) / math.sqrt(D)  # fold 1/sqrt(d) into out_wq
    vecs = np.stack([np.asarray(g1, f32), np.asarray(be1, f32),
                     np.asarray(g2, f32), np.asarray(be2, f32),
                     np.asarray(ffb1, f32), np.asarray(ffb2, f32)], axis=1)
    vecs = np.ascontiguousarray(vecs)

    shared = dict(
        wq=bf(wq_s), wk=bf(wk), wv=bf(wv), wo8=bf(wo8),
        fw1=bf(ffw1), fw2=bf(ffw2), vecs=vecs,
        owq=bf(owq_s), owk=bf(out_wk),
    )
    xT = x.transpose(0, 2, 1)                       # [B, D, LK]
    nm = (~np.asarray(mask, bool)).transpose(0, 2, 1)  # [B, LK, LQ]
    in_maps = []
    for b in range(B):
        xTf = bf(xT[b])
        nmT = bf(nm[b].astype(f32))
        for g in range(2):
            hs = h[b, g*LQL:(g+1)*LQL, :]
            in_maps.append(dict(
                shared,
                hT0=bf(hs.T), h0=np.ascontiguousarray(hs),
                xTl=bf(xT[b][:, g*LKL:(g+1)*LKL]), xTf=xTf, nmT=nmT,
            ))
    return in_maps


def kernel(**inputs):
    from concourse.bass_utils import run_bass_kernel_spmd
    nc = _get_program()
    in_maps = shard_inputs(**inputs)
    kw = {}
    if TRACE:
        import types
        try:
            import ntff_shim
            ntff_shim.install()
        except Exception:
            pass
        kw["trace"] = True
    last_err = None
    for attempt in range(3):
        try:
            res = run_bass_kernel_spmd(nc, in_maps, list(range(NCORES)), **kw)
            break
        except Exception as e:  # transient device wedges seen under axon
            last_err = e
            if attempt == 2:
                raise
    kernel.last_result = res
    out = np.empty((B, LQ, LK), np.float32)
    for b in range(B):
        for g in range(2):
            out[b, g*LQL:(g+1)*LQL, :] = res.results[b*2+g]["u"]
    return out
